# revision 21
# baseline (speedup 1.0000x reference)
"""PoissonGaussianReadout forward on 8 trn2 NeuronCores (fp8 edition).

Math (eval mode): each neuron n samples feat[b] (a [36,36,1024] image per
batch, 1024 = C*T channels) bilinearly at a fixed point mu[n], then takes a
per-neuron dot with W[n,:], adds b[n], applies elu(y)+1.

Strategy:
  - Batch-shard: 8 cores x 2 batches each; every core computes all 4096
    neurons for its 2 batches (minimal HBM bytes + minimal post-matmul work).
  - fp8 (TRN e4m3, max +-240) for x and W.  W rows are scaled to the fp8
    range; the per-neuron scale is folded into the bilinear mask, so the
    dequant is free.  Host-emulated end-to-end rel err: 1.30e-2 (<2e-2).
  - Sort neurons by bilinear base cell p00 = y0*36+x0; blocks of <=128
    sorted neurons span <= 47 cells, so a window of <= 85 flat positions
    covers all 4 bilinear corners (offsets {0,1,36,37}).
  - DoubleRow fp8 matmuls contract 256 channels per pass: 4 matmuls per
    block instead of 8, 2x PE throughput:
       psum[n, (b,j)] += sum_t Wblk[d,t,n]^T @ feat[d,t,b,wstart+j]
  - feat is staged in 2 position-halves (own SBUF tiles, contiguous DMA
    rows, 8 DMA pieces) so early blocks accumulate long before the whole
    x shard has landed.
  - PSUM groups: 3 blocks share one 2KB psum bank -> one DVE multiply
    (psum * bf16 mask) and one DVE reduce (2x bf16 mode) per group instead
    of per block.  z in bf16; bias + elu tail in f32 on ACT+DVE in 4 slices
    so output DMAs overlap the remaining compute.
  - elu(y)+1 = exp(-relu(-y)) + relu(y), exactly as the f32 baseline.
  - DMA: the two HW DGE rings (SP trigger starts streaming ~8.6us into the
    kernel, ACT trigger ~11us -- measured fixed ring bring-up) are fed in
    block-need order, balanced by predicted finish time.

The block structure / masks depend on mu, which is known when kernel() is
called; the Bass program is traced fresh per call, so they are baked in as
compile-time constants (correct for any input values).
"""
import sys
sys.path.insert(0, "/opt/trn_rl_repo")

import numpy as np
import ml_dtypes

from concourse import bass, mybir, tile
from concourse.bass_utils import run_bass_kernel_spmd
import bass_rust

# problem constants
B, C, T, HH, WW = 16, 64, 16, 36, 36
N, D = 4096, C * T            # 4096 neurons, 1024 input dim
P = HH * WW                   # 1296 flat positions
NCHUNK = 8                    # D / 128 contraction chunks
NDR = 4                       # double-row chunk pairs (256 channels each)
NCORES = 8
BPC = B // NCORES             # batches per core = 2
PAD = 38                      # max corner offset (37) + 1
WIN = 84                      # max window (even); 3*2*84*4B = 2016B <= psum bank
SPANMAX = WIN - PAD           # 47 cells of p00 span per block
GRP = 3                       # blocks per psum bank / DVE group
NTH = 2                       # feat position-halves
T0 = 660                      # half stride
TW = 744                      # half tile width (= T0 + WIN - 1)
NTAIL = 4                     # bias+elu+store slices

F32 = mybir.dt.float32
BF16 = mybir.dt.bfloat16
FP8 = mybir.dt.float8e4
E4NP = ml_dtypes.float8_e4m3  # TRN e4m3: bias 8, max +-240
BFNP = ml_dtypes.bfloat16


def _split_waits(nc, max_waits=1):
    """Walrus in this image allows only ONE sem wait per instruction.
    Hoist extra waits onto injected same-engine NoOps placed immediately
    before the owning instruction (same engine + program order => same
    semantics)."""
    k = 0
    for fn in nc.m.functions:
        for blk in fn.blocks:
            insts = blk.instructions
            out = []
            for inst in insts:
                si = inst.sync_info
                if si is not None and si.on_wait and len(si.on_wait) > max_waits:
                    waits = list(si.on_wait)
                    for w in waits[:-max_waits]:
                        nop = mybir.InstNoOp(name=f"I-wsplit-{k}", ins=[], outs=[])
                        k += 1
                        nop.engine = inst.engine
                        nop.sync_info = bass_rust.SyncInfo(
                            on_wait=[w], on_update=[]
                        )
                        out.append(nop)
                    si.on_wait = waits[-max_waits:]
                    inst.sync_info = si
                out.append(inst)
            if len(out) != len(insts):
                insts.clear()
                insts.extend(out)


def _bilinear_tables(mu):
    """Per-neuron base cell p00, corner offsets (4) in {0,1,36,37}, corner
    weights (4), replicating reference float32 arithmetic exactly."""
    one, half = np.float32(1.0), np.float32(0.5)
    g = np.clip(mu.astype(np.float32), -one, one)
    ix = (g[:, 0] + one) * np.float32(WW * 0.5) - half
    iy = (g[:, 1] + one) * np.float32(HH * 0.5) - half
    x0 = np.floor(ix)
    y0 = np.floor(iy)
    wx1 = ix - x0
    wx0 = one - wx1
    wy1 = iy - y0
    wy0 = one - wy1

    xs = [x0, x0 + one]
    ys = [y0, y0 + one]
    wxs = [wx0, wx1]
    wys = [wy0, wy1]

    x0c = np.clip(x0, 0, WW - 1).astype(np.int64)
    y0c = np.clip(y0, 0, HH - 1).astype(np.int64)
    p00 = y0c * WW + x0c

    offs = np.zeros((4, N), np.int64)
    wgts = np.zeros((4, N), np.float32)
    k = 0
    for a in range(2):          # y corner
        for bb in range(2):     # x corner
            xx, yy = xs[bb], ys[a]
            valid = (xx >= 0) & (xx <= WW - 1) & (yy >= 0) & (yy <= HH - 1)
            xi = np.clip(xx, 0, WW - 1).astype(np.int64)
            yi = np.clip(yy, 0, HH - 1).astype(np.int64)
            offs[k] = yi * WW + xi - p00
            wgts[k] = (wys[a] * wxs[bb]) * valid.astype(np.float32)
            k += 1
    assert offs.min() >= 0 and offs.max() <= 37
    return p00, offs, wgts


def _make_blocks(p00s):
    """Greedy blocks of <=128 sorted neurons spanning <= SPANMAX cells."""
    blocks = []
    s = 0
    n = len(p00s)
    while s < n:
        pfirst = p00s[s]
        e = s
        while e < n and e - s < 128 and p00s[e] - pfirst <= SPANMAX:
            e += 1
        blocks.append((s, e))
        s = e
    return blocks


def kernel(x, mu, sigma, W, b):
    x = np.ascontiguousarray(x, dtype=np.float32)
    W = np.ascontiguousarray(W, dtype=np.float32)
    b = np.asarray(b, dtype=np.float32)

    p00, offs, wgts = _bilinear_tables(mu)
    order = np.argsort(p00, kind="stable")
    p00s = p00[order]
    blocks = _make_blocks(p00s)
    nblk = len(blocks)
    # pad block widths to a multiple of 16 (dual-fp8 ldweights needs the
    # k-tile step, = m bytes, 16B aligned)
    ms = [-(-(e - s) // 16) * 16 for s, e in blocks]

    # groups of GRP blocks per psum bank; taper the LAST groups down to
    # single blocks so the post-DMA drain pipeline ends in small steps.
    head = [1, 2]
    taper = [2, 2, 1, 1]
    body = nblk - sum(taper) - sum(head)
    gsizes = list(head) + [GRP] * (body // GRP)
    if body % GRP:
        gsizes.append(body % GRP)
    gsizes += taper
    gbounds = np.cumsum([0] + gsizes)
    ngrp = len(gsizes)

    # ---- host packing ----
    # per-neuron fp8 row scale, folded into the mask
    scale = (np.abs(W).max(axis=1) / np.float32(240.0)).astype(np.float32)
    scale = np.maximum(scale, np.float32(1e-30))
    Wq = (W / scale[:, None]).astype(E4NP)            # [N, D] fp8

    # ragged W pack: per block [dr, t, m] flattened; per-group column range
    boffs = np.cumsum([0] + [NCHUNK * m for m in ms])
    totc = int(boffs[-1])
    wf = np.zeros((128, totc), E4NP)
    mk = np.zeros((128, nblk, WIN), np.float32)
    bt = np.zeros((128, nblk, 2), np.float32)
    wstarts, spans = [], []
    for i, (s, e) in enumerate(blocks):
        idx = order[s:e]
        m = e - s
        pfirst = int(p00s[s])
        wstarts.append(pfirst)
        spans.append(int(p00s[e - 1]) - pfirst)
        # [m, D] -> [c(8), p(128), m] -> [p, (dr, t, m_pad)]
        mp = ms[i]
        blk = np.ascontiguousarray(Wq[idx].T).reshape(NCHUNK, 128, m)
        arr = np.zeros((128, NCHUNK, mp), E4NP)
        arr[:, :, :m] = blk.transpose(1, 0, 2)
        wf[:, int(boffs[i]):int(boffs[i + 1])] = arr.reshape(128, NCHUNK * mp)
        rel = p00[idx] - pfirst                        # [m] in 0..SPANMAX
        for k in range(4):
            np.add.at(mk[:m, i], (np.arange(m), rel + offs[k][idx]),
                      wgts[k][idx] * scale[idx])
        bt[:m, i, 0] = b[idx]
        bt[:m, i, 1] = b[idx]
    mf = mk.astype(BFNP)

    # per-block feat half + in-half window start; per-group window width
    tht = [ws // T0 for ws in wstarts]
    assert all(ws - T0 * t + WIN <= TW for ws, t in zip(wstarts, tht))
    gwin = [min(WIN, 2 * ((max(spans[i] for i in range(gbounds[g], gbounds[g + 1]))
                              + PAD + 1) // 2))
            for g in range(ngrp)]
    gthird = [tht[int(gbounds[g + 1]) - 1] for g in range(ngrp)]

    # tail slices (block ranges); final two tails cover single small groups
    tcuts = sorted({max(1, round((ngrp - 2) * k / NTAIL)) for k in range(1, NTAIL + 1)}
                   | {ngrp - 1, ngrp})
    tails = []
    prev = 0
    for cut in tcuts:
        tails.append((cut - 1, int(gbounds[prev]), int(gbounds[cut])))
        prev = cut
    tail_after = {g: (lo, hi) for g, lo, hi in tails}

    # ---- build the Bass program (same for all cores) ----
    nc = bass.Bass()
    xs_h = nc.declare_dram_parameter("xs", [128, NTH, NDR, 2, BPC, TW], FP8,
                                     isOutput=False)
    wf_h = nc.declare_dram_parameter("wf", [128, totc], FP8, isOutput=False)
    mf_h = nc.declare_dram_parameter("mf", [128, nblk, WIN], BF16,
                                     isOutput=False)
    bt_h = nc.declare_dram_parameter("bt", [128, nblk, 2], F32, isOutput=False)
    z_h = nc.declare_dram_parameter("z", [128, nblk, 2], F32, isOutput=True)

    with tile.TileContext(nc) as tc:
        with (
            tc.tile_pool(name="feat", bufs=1) as featp,
            tc.tile_pool(name="wpool", bufs=1) as wpool,
            tc.tile_pool(name="mpool", bufs=1) as mpool,
            tc.tile_pool(name="spool", bufs=4) as spool,
            tc.tile_pool(name="zpool", bufs=1) as zpool,
            tc.tile_pool(name="psum", bufs=1, space="PSUM") as psump,
        ):
            f3 = [featp.tile([128, NDR, 2, BPC, TW], FP8, name=f"f3_{t}")
                  for t in range(NTH)]
            wgs = [wpool.tile(
                [128, int(boffs[gbounds[g + 1]] - boffs[gbounds[g]])], FP8,
                name=f"wg{g}") for g in range(ngrp)]
            mask_t = mpool.tile([128, nblk, WIN], BF16)
            bt_t = zpool.tile([128, nblk, 2], F32)
            zAll = zpool.tile([128, nblk, 2], BF16)
            yt = zpool.tile([128, nblk, 2], F32)
            rp = zpool.tile([128, nblk, 2], F32)
            rn = zpool.tile([128, nblk, 2], F32)
            ep = zpool.tile([128, nblk, 2], F32)
            ot = zpool.tile([128, nblk, 2], F32)

            # ---- DMA items in block-need order; assign to the two HW DGE
            # rings (sync streams from ~8.6us, scalar from ~10.2us --
            # measured ring bring-up) by predicted finish time ----
            def wg_item(g):
                c0 = int(boffs[gbounds[g]])
                c1 = int(boffs[gbounds[g + 1]])
                return (wgs[g][:], wf_h[:, c0:c1])

            # mask pieces (tiny first) with the group index that first needs
            # each, so the first drains aren't gated on a big mask DMA
            cuts = sorted({0, int(gbounds[2]), int(round(nblk * 0.4)),
                           int(round(nblk * 0.7)), nblk})
            mfq = []
            for k in range(len(cuts) - 1):
                lo, hi = cuts[k], cuts[k + 1]
                if hi > lo:
                    need_g = next(g for g in range(ngrp)
                                  if gbounds[g + 1] > lo)
                    mfq.append((need_g, (mask_t[:, lo:hi], mf_h[:, lo:hi])))
            f1q = [(f3[1][:, p], xs_h[:, 1, p]) for p in range(NDR)]

            items = [(f3[0][:, p], xs_h[:, 0, p]) for p in range(NDR)]
            for g in range(ngrp):
                while mfq and mfq[0][0] <= g + 1:
                    items.append(mfq.pop(0)[1])
                if gthird[g] > 0:
                    items.extend(f1q)
                    f1q = []
                items.append(wg_item(g))
                if g == 1:
                    items.append((bt_t[:], bt_h[:]))
                if g >= 2 and f1q:
                    items.append(f1q.pop(0))
            items.extend(f1q)

            # measured ring characteristics: sync streams from ~8.7us at
            # ~183 GB/s, scalar from ~10.1us at ~204 GB/s
            ta, tb = 8.7, 10.1
            ra, rb = 0.183, 0.204
            for dst, src in items:
                nb = src.nbytes() / 1e6
                if ta + nb / ra <= tb + nb / rb:
                    nc.sync.dma_start(dst, src)
                    ta += nb / ra
                else:
                    nc.scalar.dma_start(dst, src)
                    tb += nb / rb

            def tail(lo, hi, last=False):
                # y = z + bias ; out = exp(-relu(-y)) + relu(y)
                # (relu/exp on ACT, the adds/max on GpSimd to keep DVE free
                # for the psum drains; the final store rides the idle DVE
                # to dodge sync-queue serialization)
                sl = slice(lo, hi)
                nc.vector.tensor_add(yt[:, sl], zAll[:, sl], bt_t[:, sl])
                nc.scalar.activation(rn[:, sl], yt[:, sl],
                                     mybir.ActivationFunctionType.Relu,
                                     scale=-1.0)
                nc.gpsimd.tensor_scalar_max(rp[:, sl], yt[:, sl], 0.0)
                nc.scalar.activation(ep[:, sl], rn[:, sl],
                                     mybir.ActivationFunctionType.Exp,
                                     scale=-1.0)
                nc.gpsimd.tensor_add(ot[:, sl], ep[:, sl], rp[:, sl])
                eng = nc.scalar if last else nc.sync
                eng.dma_start(z_h[:, sl], ot[:, sl])

            for g in range(ngrp):
                i0, i1 = int(gbounds[g]), int(gbounds[g + 1])
                gsz = i1 - i0
                wg_ = gwin[g]
                gc0 = int(boffs[i0])
                pm = psump.tile([128, gsz, 2, WIN], F32, name=f"pm{g}",
                                tag=f"bank{g % 8}")
                for j in range(gsz):
                    i = i0 + j
                    m = ms[i]
                    t = tht[i]
                    ws = wstarts[i] - T0 * t
                    o = int(boffs[i]) - gc0
                    wb = wgs[g][:, o:o + NCHUNK * m].rearrange(
                        "p (dr t m) -> p dr t m", dr=NDR, t=2)
                    for dr in range(NDR):
                        nc.tensor.matmul(
                            pm[0:m, j, :, 0:wg_],
                            wb[:, dr],
                            f3[t][:, dr, :, :, ws:ws + wg_],
                            start=(dr == 0),
                            stop=(dr == NDR - 1),
                            perf_mode=mybir.MatmulPerfMode.DoubleRow,
                        )
                masked = spool.tile([128, GRP, 2, WIN], BF16,
                                    tag=f"mx{g % 4}")
                mkb = mask_t[:, i0:i1, 0:wg_].unsqueeze(2).broadcast_to(
                    (128, gsz, 2, wg_))
                nc.vector.tensor_mul(masked[:, 0:gsz, :, 0:wg_],
                                     pm[:, :, :, 0:wg_], mkb)
                with nc.allow_low_precision("bf16 z; host-verified 1.3e-2"):
                    nc.vector.tensor_reduce(
                        zAll[:, i0:i1], masked[:, 0:gsz, :, 0:wg_],
                        axis=mybir.AxisListType.X,
                        op=mybir.AluOpType.add,
                    )
                if g in tail_after:
                    tail(*tail_after[g], last=(g == ngrp - 1))

    _split_waits(nc)

    # ---- run on 8 cores ----
    # xs host layout [128, half, pair, ktile, b, TW]: chunk c = 2*pair+ktile,
    # channel d = c*128 + partition; half t covers positions [T0*t, T0*t+TW).
    xq = x.reshape(B, NDR, 2, 128, P).astype(E4NP)
    in_maps = []
    for core in range(NCORES):
        xs_dev = np.zeros((128, NTH, NDR, 2, BPC, TW), E4NP)
        for t in range(NTH):
            lo = T0 * t
            hi = min(P, lo + TW)
            xs_dev[:, t, :, :, :, :hi - lo] = (
                xq[BPC * core:BPC * (core + 1), :, :, :, lo:hi]
                .transpose(3, 1, 2, 0, 4)
            )
        in_maps.append({
            "xs": xs_dev,
            "wf": wf,
            "mf": mf,
            "bt": bt,
        })
    res = run_bass_kernel_spmd(nc, in_maps, core_ids=list(range(NCORES)))

    # ---- assemble ----
    y = np.empty((B, N), np.float32)
    for core in range(NCORES):
        z = res.results[core]["z"]
        for i, (s, e) in enumerate(blocks):
            idx = order[s:e]
            m = e - s
            y[BPC * core, idx] = z[0:m, i, 0]
            y[BPC * core + 1, idx] = z[0:m, i, 1]
    return y


# revision 22
# speedup vs baseline: 1.0342x; 1.0342x over previous
"""PoissonGaussianReadout forward on 8 trn2 NeuronCores (fp8 edition).

Math (eval mode): each neuron n samples feat[b] (a [36,36,1024] image per
batch, 1024 = C*T channels) bilinearly at a fixed point mu[n], then takes a
per-neuron dot with W[n,:], adds b[n], applies elu(y)+1.

Strategy:
  - Batch-shard: 8 cores x 2 batches each; every core computes all 4096
    neurons for its 2 batches (minimal HBM bytes + minimal post-matmul work).
  - fp8 (TRN e4m3, max +-240) for x and W.  W rows are scaled to the fp8
    range; the per-neuron scale is folded into the bilinear mask, so the
    dequant is free.  Host-emulated end-to-end rel err: 1.30e-2 (<2e-2).
  - Sort neurons by bilinear base cell p00 = y0*36+x0; blocks of <=128
    sorted neurons span <= 47 cells, so a window of <= 85 flat positions
    covers all 4 bilinear corners (offsets {0,1,36,37}).
  - DoubleRow fp8 matmuls contract 256 channels per pass: 4 matmuls per
    block instead of 8, 2x PE throughput:
       psum[n, (b,j)] += sum_t Wblk[d,t,n]^T @ feat[d,t,b,wstart+j]
  - feat is staged in 2 position-halves (own SBUF tiles, contiguous DMA
    rows, 8 DMA pieces) so early blocks accumulate long before the whole
    x shard has landed.
  - PSUM groups: 3 blocks share one 2KB psum bank -> one DVE multiply
    (psum * bf16 mask) and one DVE reduce (2x bf16 mode) per group instead
    of per block.  z in bf16; bias + elu tail in f32 on ACT+DVE in 4 slices
    so output DMAs overlap the remaining compute.
  - elu(y)+1 = exp(-relu(-y)) + relu(y), exactly as the f32 baseline.
  - DMA: the two HW DGE rings (SP trigger starts streaming ~8.6us into the
    kernel, ACT trigger ~11us -- measured fixed ring bring-up) are fed in
    block-need order, balanced by predicted finish time.

The block structure / masks depend on mu, which is known when kernel() is
called; the Bass program is traced fresh per call, so they are baked in as
compile-time constants (correct for any input values).
"""
import sys
sys.path.insert(0, "/opt/trn_rl_repo")

import numpy as np
import ml_dtypes

from concourse import bass, mybir, tile
from concourse.bass_utils import run_bass_kernel_spmd
import bass_rust

# problem constants
B, C, T, HH, WW = 16, 64, 16, 36, 36
N, D = 4096, C * T            # 4096 neurons, 1024 input dim
P = HH * WW                   # 1296 flat positions
NCHUNK = 8                    # D / 128 contraction chunks
NDR = 4                       # double-row chunk pairs (256 channels each)
NCORES = 8
BPC = B // NCORES             # batches per core = 2
PAD = 38                      # max corner offset (37) + 1
WIN = 84                      # max window (even); 3*2*84*4B = 2016B <= psum bank
SPANMAX = WIN - PAD           # 47 cells of p00 span per block
GRP = 3                       # blocks per psum bank / DVE group
NTH = 2                       # feat position-halves
T0 = 660                      # half stride
TW = 744                      # half tile width (= T0 + WIN - 1)
NTAIL = 4                     # bias+elu+store slices

F32 = mybir.dt.float32
BF16 = mybir.dt.bfloat16
FP8 = mybir.dt.float8e4
E4NP = ml_dtypes.float8_e4m3  # TRN e4m3: bias 8, max +-240
BFNP = ml_dtypes.bfloat16


def _split_waits(nc, max_waits=1):
    """Walrus in this image allows only ONE sem wait per instruction.
    Hoist extra waits onto injected same-engine NoOps placed immediately
    before the owning instruction (same engine + program order => same
    semantics)."""
    k = 0
    for fn in nc.m.functions:
        for blk in fn.blocks:
            insts = blk.instructions
            out = []
            for inst in insts:
                si = inst.sync_info
                if si is not None and si.on_wait and len(si.on_wait) > max_waits:
                    waits = list(si.on_wait)
                    for w in waits[:-max_waits]:
                        nop = mybir.InstNoOp(name=f"I-wsplit-{k}", ins=[], outs=[])
                        k += 1
                        nop.engine = inst.engine
                        nop.sync_info = bass_rust.SyncInfo(
                            on_wait=[w], on_update=[]
                        )
                        out.append(nop)
                    si.on_wait = waits[-max_waits:]
                    inst.sync_info = si
                out.append(inst)
            if len(out) != len(insts):
                insts.clear()
                insts.extend(out)


def _bilinear_tables(mu):
    """Per-neuron base cell p00, corner offsets (4) in {0,1,36,37}, corner
    weights (4), replicating reference float32 arithmetic exactly."""
    one, half = np.float32(1.0), np.float32(0.5)
    g = np.clip(mu.astype(np.float32), -one, one)
    ix = (g[:, 0] + one) * np.float32(WW * 0.5) - half
    iy = (g[:, 1] + one) * np.float32(HH * 0.5) - half
    x0 = np.floor(ix)
    y0 = np.floor(iy)
    wx1 = ix - x0
    wx0 = one - wx1
    wy1 = iy - y0
    wy0 = one - wy1

    xs = [x0, x0 + one]
    ys = [y0, y0 + one]
    wxs = [wx0, wx1]
    wys = [wy0, wy1]

    x0c = np.clip(x0, 0, WW - 1).astype(np.int64)
    y0c = np.clip(y0, 0, HH - 1).astype(np.int64)
    p00 = y0c * WW + x0c

    offs = np.zeros((4, N), np.int64)
    wgts = np.zeros((4, N), np.float32)
    k = 0
    for a in range(2):          # y corner
        for bb in range(2):     # x corner
            xx, yy = xs[bb], ys[a]
            valid = (xx >= 0) & (xx <= WW - 1) & (yy >= 0) & (yy <= HH - 1)
            xi = np.clip(xx, 0, WW - 1).astype(np.int64)
            yi = np.clip(yy, 0, HH - 1).astype(np.int64)
            offs[k] = yi * WW + xi - p00
            wgts[k] = (wys[a] * wxs[bb]) * valid.astype(np.float32)
            k += 1
    assert offs.min() >= 0 and offs.max() <= 37
    return p00, offs, wgts


def _make_blocks(p00s):
    """Greedy blocks of <=128 sorted neurons spanning <= SPANMAX cells."""
    blocks = []
    s = 0
    n = len(p00s)
    while s < n:
        pfirst = p00s[s]
        e = s
        while e < n and e - s < 128 and p00s[e] - pfirst <= SPANMAX:
            e += 1
        blocks.append((s, e))
        s = e
    return blocks


def kernel(x, mu, sigma, W, b):
    x = np.ascontiguousarray(x, dtype=np.float32)
    W = np.ascontiguousarray(W, dtype=np.float32)
    b = np.asarray(b, dtype=np.float32)

    p00, offs, wgts = _bilinear_tables(mu)
    order = np.argsort(p00, kind="stable")
    p00s = p00[order]
    blocks = _make_blocks(p00s)
    nblk = len(blocks)
    # pad block widths to a multiple of 16 (dual-fp8 ldweights needs the
    # k-tile step, = m bytes, 16B aligned)
    ms = [-(-(e - s) // 16) * 16 for s, e in blocks]

    # groups of GRP blocks per psum bank; taper the LAST groups down to
    # single blocks so the post-DMA drain pipeline ends in small steps.
    head = [1, 2]
    taper = [2, 2, 1, 1]
    body = nblk - sum(taper) - sum(head)
    gsizes = list(head) + [GRP] * (body // GRP)
    if body % GRP:
        gsizes.append(body % GRP)
    gsizes += taper
    gbounds = np.cumsum([0] + gsizes)
    ngrp = len(gsizes)

    # ---- host packing ----
    # per-neuron fp8 row scale, folded into the mask
    scale = (np.abs(W).max(axis=1) / np.float32(240.0)).astype(np.float32)
    scale = np.maximum(scale, np.float32(1e-30))
    Wq = (W / scale[:, None]).astype(E4NP)            # [N, D] fp8

    # ragged W pack: per block [dr, t, m] flattened; per-group column range
    boffs = np.cumsum([0] + [NCHUNK * m for m in ms])
    totc = int(boffs[-1])
    wf = np.zeros((128, totc), E4NP)
    mk = np.zeros((128, nblk, WIN), np.float32)
    bt = np.zeros((128, nblk, 2), np.float32)
    wstarts, spans = [], []
    for i, (s, e) in enumerate(blocks):
        idx = order[s:e]
        m = e - s
        pfirst = int(p00s[s])
        wstarts.append(pfirst)
        spans.append(int(p00s[e - 1]) - pfirst)
        # [m, D] -> [c(8), p(128), m] -> [p, (dr, t, m_pad)]
        mp = ms[i]
        blk = np.ascontiguousarray(Wq[idx].T).reshape(NCHUNK, 128, m)
        arr = np.zeros((128, NCHUNK, mp), E4NP)
        arr[:, :, :m] = blk.transpose(1, 0, 2)
        wf[:, int(boffs[i]):int(boffs[i + 1])] = arr.reshape(128, NCHUNK * mp)
        rel = p00[idx] - pfirst                        # [m] in 0..SPANMAX
        for k in range(4):
            np.add.at(mk[:m, i], (np.arange(m), rel + offs[k][idx]),
                      wgts[k][idx] * scale[idx])
        bt[:m, i, 0] = b[idx]
        bt[:m, i, 1] = b[idx]
    mf = mk.astype(BFNP)

    # per-block feat half + in-half window start; per-group window width
    tht = [ws // T0 for ws in wstarts]
    assert all(ws - T0 * t + WIN <= TW for ws, t in zip(wstarts, tht))
    gwin = [min(WIN, 2 * ((max(spans[i] for i in range(gbounds[g], gbounds[g + 1]))
                              + PAD + 1) // 2))
            for g in range(ngrp)]
    gthird = [tht[int(gbounds[g + 1]) - 1] for g in range(ngrp)]

    # tail slices (block ranges); final two tails cover single small groups
    tcuts = sorted({max(1, round((ngrp - 2) * k / NTAIL)) for k in range(1, NTAIL + 1)}
                   | {ngrp - 1, ngrp})
    tails = []
    prev = 0
    for cut in tcuts:
        tails.append((cut - 1, int(gbounds[prev]), int(gbounds[cut])))
        prev = cut
    tail_after = {g: (lo, hi) for g, lo, hi in tails}

    # ---- build the Bass program (same for all cores) ----
    nc = bass.Bass()
    xs_h = nc.declare_dram_parameter("xs", [128, NTH, NDR, 2, BPC, TW], FP8,
                                     isOutput=False)
    wf_h = nc.declare_dram_parameter("wf", [128, totc], FP8, isOutput=False)
    mf_h = nc.declare_dram_parameter("mf", [128, nblk, WIN], BF16,
                                     isOutput=False)
    bt_h = nc.declare_dram_parameter("bt", [128, nblk, 2], F32, isOutput=False)
    z_h = nc.declare_dram_parameter("z", [128, nblk, 2], F32, isOutput=True)

    with tile.TileContext(nc) as tc:
        with (
            tc.tile_pool(name="feat", bufs=1) as featp,
            tc.tile_pool(name="wpool", bufs=1) as wpool,
            tc.tile_pool(name="mpool", bufs=1) as mpool,
            tc.tile_pool(name="spool", bufs=4) as spool,
            tc.tile_pool(name="zpool", bufs=1) as zpool,
            tc.tile_pool(name="psum", bufs=1, space="PSUM") as psump,
        ):
            f3 = [featp.tile([128, NDR, 2, BPC, TW], FP8, name=f"f3_{t}")
                  for t in range(NTH)]
            wgs = [wpool.tile(
                [128, int(boffs[gbounds[g + 1]] - boffs[gbounds[g]])], FP8,
                name=f"wg{g}") for g in range(ngrp)]
            mask_t = mpool.tile([128, nblk, WIN], BF16)
            bt_t = zpool.tile([128, nblk, 2], F32)
            zAll = zpool.tile([128, nblk, 2], BF16)
            yt = zpool.tile([128, nblk, 2], F32)
            rp = zpool.tile([128, nblk, 2], F32)
            rn = zpool.tile([128, nblk, 2], F32)
            ep = zpool.tile([128, nblk, 2], F32)
            ot = zpool.tile([128, nblk, 2], F32)

            # ---- DMA items in block-need order; assign to the two HW DGE
            # rings (sync streams from ~8.6us, scalar from ~10.2us --
            # measured ring bring-up) by predicted finish time ----
            def wg_item(g):
                c0 = int(boffs[gbounds[g]])
                c1 = int(boffs[gbounds[g + 1]])
                return (wgs[g][:], wf_h[:, c0:c1])

            # mask pieces (tiny first) with the group index that first needs
            # each, so the first drains aren't gated on a big mask DMA
            cuts = sorted({0, int(gbounds[2]), int(round(nblk * 0.4)),
                           int(round(nblk * 0.7)), nblk})
            mfq = []
            for k in range(len(cuts) - 1):
                lo, hi = cuts[k], cuts[k + 1]
                if hi > lo:
                    need_g = next(g for g in range(ngrp)
                                  if gbounds[g + 1] > lo)
                    mfq.append((need_g, (mask_t[:, lo:hi], mf_h[:, lo:hi])))
            f1q = [(f3[1][:, p], xs_h[:, 1, p]) for p in range(NDR)]

            items = [(f3[0][:, p], xs_h[:, 0, p]) for p in range(NDR)]
            for g in range(ngrp):
                while mfq and mfq[0][0] <= g + 1:
                    items.append(mfq.pop(0)[1])
                if gthird[g] > 0:
                    items.extend(f1q)
                    f1q = []
                items.append(wg_item(g))
                if g == 1:
                    items.append((bt_t[:], bt_h[:]))
                if g >= 2 and f1q:
                    items.append(f1q.pop(0))
            items.extend(f1q)

            # the two HW DGE rings start streaming in nondeterministic order
            # (~8.7us / ~10.1us), so balance purely by bytes -- robust to
            # either ordering
            ta, tb = 0.0, 0.0
            for dst, src in items:
                nb = src.nbytes() / 1e6
                if ta <= tb:
                    nc.sync.dma_start(dst, src)
                    ta += nb
                else:
                    nc.scalar.dma_start(dst, src)
                    tb += nb

            def tail(lo, hi, last=False):
                # y = z + bias ; out = exp(-relu(-y)) + relu(y)
                # (relu/exp on ACT, the adds/max on GpSimd to keep DVE free
                # for the psum drains; the final store rides the idle DVE
                # to dodge sync-queue serialization)
                sl = slice(lo, hi)
                nc.vector.tensor_add(yt[:, sl], zAll[:, sl], bt_t[:, sl])
                nc.scalar.activation(rn[:, sl], yt[:, sl],
                                     mybir.ActivationFunctionType.Relu,
                                     scale=-1.0)
                nc.gpsimd.tensor_scalar_max(rp[:, sl], yt[:, sl], 0.0)
                nc.scalar.activation(ep[:, sl], rn[:, sl],
                                     mybir.ActivationFunctionType.Exp,
                                     scale=-1.0)
                nc.gpsimd.tensor_add(ot[:, sl], ep[:, sl], rp[:, sl])
                eng = nc.scalar if last else nc.sync
                eng.dma_start(z_h[:, sl], ot[:, sl])

            for g in range(ngrp):
                i0, i1 = int(gbounds[g]), int(gbounds[g + 1])
                gsz = i1 - i0
                wg_ = gwin[g]
                gc0 = int(boffs[i0])
                pm = psump.tile([128, gsz, 2, WIN], F32, name=f"pm{g}",
                                tag=f"bank{g % 8}")
                for j in range(gsz):
                    i = i0 + j
                    m = ms[i]
                    t = tht[i]
                    ws = wstarts[i] - T0 * t
                    o = int(boffs[i]) - gc0
                    wb = wgs[g][:, o:o + NCHUNK * m].rearrange(
                        "p (dr t m) -> p dr t m", dr=NDR, t=2)
                    for dr in range(NDR):
                        nc.tensor.matmul(
                            pm[0:m, j, :, 0:wg_],
                            wb[:, dr],
                            f3[t][:, dr, :, :, ws:ws + wg_],
                            start=(dr == 0),
                            stop=(dr == NDR - 1),
                            perf_mode=mybir.MatmulPerfMode.DoubleRow,
                        )
                masked = spool.tile([128, GRP, 2, WIN], BF16,
                                    tag=f"mx{g % 4}")
                mkb = mask_t[:, i0:i1, 0:wg_].unsqueeze(2).broadcast_to(
                    (128, gsz, 2, wg_))
                nc.vector.tensor_mul(masked[:, 0:gsz, :, 0:wg_],
                                     pm[:, :, :, 0:wg_], mkb)
                with nc.allow_low_precision("bf16 z; host-verified 1.3e-2"):
                    nc.vector.tensor_reduce(
                        zAll[:, i0:i1], masked[:, 0:gsz, :, 0:wg_],
                        axis=mybir.AxisListType.X,
                        op=mybir.AluOpType.add,
                    )
                if g in tail_after:
                    tail(*tail_after[g], last=(g == ngrp - 1))

    _split_waits(nc)

    # ---- run on 8 cores ----
    # xs host layout [128, half, pair, ktile, b, TW]: chunk c = 2*pair+ktile,
    # channel d = c*128 + partition; half t covers positions [T0*t, T0*t+TW).
    xq = x.reshape(B, NDR, 2, 128, P).astype(E4NP)
    in_maps = []
    for core in range(NCORES):
        xs_dev = np.zeros((128, NTH, NDR, 2, BPC, TW), E4NP)
        for t in range(NTH):
            lo = T0 * t
            hi = min(P, lo + TW)
            xs_dev[:, t, :, :, :, :hi - lo] = (
                xq[BPC * core:BPC * (core + 1), :, :, :, lo:hi]
                .transpose(3, 1, 2, 0, 4)
            )
        in_maps.append({
            "xs": xs_dev,
            "wf": wf,
            "mf": mf,
            "bt": bt,
        })
    res = run_bass_kernel_spmd(nc, in_maps, core_ids=list(range(NCORES)))

    # ---- assemble ----
    y = np.empty((B, N), np.float32)
    for core in range(NCORES):
        z = res.results[core]["z"]
        for i, (s, e) in enumerate(blocks):
            idx = order[s:e]
            m = e - s
            y[BPC * core, idx] = z[0:m, i, 0]
            y[BPC * core + 1, idx] = z[0:m, i, 1]
    return y
